# revision 1
# baseline (speedup 1.0000x reference)
"""Causal self-attention (B=4, T=2048, D=1024, H=16) on 8 NeuronCores.

Sharding: core c handles batch b=c//2 and head-group hg=c%2 (8 of 16 heads).
Per core: column-parallel Wq/Wk/Wv (512 cols), row-parallel Wo (512 rows).
Host sums the two partial outputs per batch and adds bo_eff. No collectives.

Structure (v3):
  - x^T resident in SBUF (bf16), loaded once with block DMAs spread over queues.
  - All biases eliminated from the kernel math:
      * bk and per-query additive score terms drop (softmax invariance).
      * bq.k_raw folds in MULTIPLICATIVELY: V rows (and the sumexp ones
        column) are scaled by ebqk = exp(0.125*bq.k_raw) during the V phase
        (idle-ACT per-partition scale instrs); exp then needs no bias at all.
      * bv commutes through attention; host folds bv@Wo into bo.
  - Scores: both heads write ONE [128,1024] 2-bank PSUM tile (row-group
    packed, K=64 each, equal deps -> adjacent issue -> PE-concurrent).
    One exp instr covers both heads. Causal triangle zeroed post-exp with a
    0/1 bf16 mask on DVE.
  - PV accumulates unnormalized out^T + scaled-sumexp row (ones column, M=65).
  - Division per q-block: copy d row out of PSUM, reciprocal_approx_fast,
    bf16 cast, two K=1 selector matmuls broadcast 1/d, one DVE multiply.
  - Weight-load pairing: projection steps process two 512-blocks per chunk
    load; O-proj processes both output halves per oat load.
  - Next-pair Q/K projections fill the PE during attention; pair 3 interleaves
    the final projection for already-finished q-blocks instead.
"""

import os
from contextlib import ExitStack

import ml_dtypes
import numpy as np

import concourse.bacc as bacc
import concourse.mybir as mybir
import concourse.tile as tile
from concourse.bass_utils import run_bass_kernel_spmd

B, T, D, H, DK = 4, 2048, 1024, 16, 64
HL = 8  # heads per core
CD = HL * DK  # 512 local channels
NP = 128  # partitions
QB = 512  # query block
NDC = D // NP  # 8 din chunks
NTT = T // NP  # 16 t-tiles
NTB = T // QB  # 4 t-blocks
NPAIR = HL // 2  # 4 head pairs
VW = DK + 2  # v row stride (64 data + 1 ones + 1 pad for 4B alignment)
F32 = mybir.dt.float32
BF16 = mybir.dt.bfloat16
Exp = mybir.ActivationFunctionType.Exp
Identity = mybir.ActivationFunctionType.Identity

_CACHE: dict = {}


def _build_nc():
    nc = bacc.Bacc("TRN2", target_bir_lowering=False, debug=False)
    xt = nc.dram_tensor("xt", [D, T], BF16, kind="ExternalInput")
    wq = nc.dram_tensor("wq", [D, CD], BF16, kind="ExternalInput")
    wk = nc.dram_tensor("wk", [D, CD], BF16, kind="ExternalInput")
    wv = nc.dram_tensor("wv", [D, CD], BF16, kind="ExternalInput")
    wkb = nc.dram_tensor("wkb", [D, HL], BF16, kind="ExternalInput")
    wo = nc.dram_tensor("wo", [CD, D], BF16, kind="ExternalInput")
    tri = nc.dram_tensor("tri", [NP, NP], BF16, kind="ExternalInput")
    y = nc.dram_tensor("y", [T, D], BF16, kind="ExternalOutput")

    with tile.TileContext(nc) as tc, ExitStack() as ctx:
        _body(nc, tc, ctx, xt, wq, wk, wv, wkb, wo, tri, y)
    nc.compile()
    return nc


def _body(nc, tc, ctx, xt, wq, wk, wv, wkb, wo, tri, y):
    const = ctx.enter_context(tc.tile_pool(name="const", bufs=1))
    xtp = ctx.enter_context(tc.tile_pool(name="xt", bufs=1))
    vpool = ctx.enter_context(tc.tile_pool(name="v", bufs=1))
    ebqkp = ctx.enter_context(tc.tile_pool(name="ebqk", bufs=1))
    oatp = ctx.enter_context(tc.tile_pool(name="oat", bufs=1))
    wqkp = ctx.enter_context(tc.tile_pool(name="wqk", bufs=2))
    qkp = ctx.enter_context(tc.tile_pool(name="qk", bufs=2))
    etp = ctx.enter_context(tc.tile_pool(name="et", bufs=2))
    pvsp = ctx.enter_context(tc.tile_pool(name="pvs", bufs=2))
    dnp = ctx.enter_context(tc.tile_pool(name="dn", bufs=2))
    smallp = ctx.enter_context(tc.tile_pool(name="small", bufs=2))
    wop = ctx.enter_context(tc.tile_pool(name="wop", bufs=1))
    # PSUM: proj(2) always; V phase adds bqkps(2); attention adds s(2x2)+pv(2)
    projps = ctx.enter_context(tc.tile_pool(name="projps", bufs=2, space="PSUM"))

    # ---- constants ----
    tri_sb = const.tile([NP, NP], BF16, tag="tri")
    nc.scalar.dma_start(tri_sb[:], tri[:])
    sel64 = []
    for h in range(2):
        st = const.tile([1, NP], BF16, tag=f"sel{h}", name=f"sel{h}")
        nc.vector.memset(st[:], 0.0)
        nc.vector.memset(st[0:1, 64 * h : 64 * h + 64], 1.0)
        sel64.append(st)
    warm = const.tile([1, 2], F32, tag="warm")
    nc.vector.memset(warm[:], 0.0)
    nc.scalar.activation(warm[:], warm[:], Exp)

    # ---- resident x^T (bf16): block DMAs, t-block major so V can start early
    xt_sb = xtp.tile([NP, NDC, T], BF16, tag="xt")
    for tb in range(NTB):
        bsl = slice(tb * QB, (tb + 1) * QB)
        for d in range(NDC):
            nc.sync.dma_start(xt_sb[:, d, bsl], xt[d * NP : (d + 1) * NP, bsl])

    # ---- V phase weights on the vector queue (parallel with xt on sync) ----
    v_sb = [
        vpool.tile([NP, HL, VW], BF16, tag=f"v{tt}", name=f"v{tt}")
        for tt in range(NTT)
    ]
    ebqk_sb = ebqkp.tile([NP, NTT, HL], F32, tag="ebqk")
    wvp_cm = tc.tile_pool(name="wvp", bufs=1)
    wvp = wvp_cm.__enter__()
    bqkps_cm = tc.tile_pool(name="bqkps", bufs=2, space="PSUM")
    bqkps = bqkps_cm.__enter__()
    wv_sb = wvp.tile([NP, NDC, CD], BF16, tag="wv")
    wkb_sb = wvp.tile([NP, NDC, HL], BF16, tag="wkb")
    for d in range(NDC):
        nc.scalar.dma_start(wv_sb[:, d, :], wv[d * NP : (d + 1) * NP, :])
        nc.scalar.dma_start(wkb_sb[:, d, :], wkb[d * NP : (d + 1) * NP, :])

    wq_t = [None] * NPAIR
    wk_t = [None] * NPAIR

    def emit_wqk_dma(c):
        wq_t[c] = wqkp.tile([NP, NDC, NP], BF16, tag="wqc", name="wqc")
        wk_t[c] = wqkp.tile([NP, NDC, NP], BF16, tag="wkc", name="wkc")
        for d in range(NDC):
            nc.gpsimd.dma_start(
                wq_t[c][:, d, :], wq[d * NP : (d + 1) * NP, c * NP : (c + 1) * NP]
            )
            nc.gpsimd.dma_start(
                wk_t[c][:, d, :], wk[d * NP : (d + 1) * NP, c * NP : (c + 1) * NP]
            )

    emit_wqk_dma(0)

    # ---- V phase: V[t, h, dv]*ebqk (bf16) + scaled ones col ----
    for tt in range(NTT):
        tsl = slice(tt * NP, (tt + 1) * NP)
        psv = projps.tile([NP, CD], F32, tag="proj")
        psb = bqkps.tile([NP, HL], F32, tag="bqkps")
        for d in range(NDC):
            nc.tensor.matmul(
                psv[:], xt_sb[:, d, tsl], wv_sb[:, d, :],
                start=(d == 0), stop=(d == NDC - 1),
            )
            nc.tensor.matmul(
                psb[:], xt_sb[:, d, tsl], wkb_sb[:, d, :],
                start=(d == 0), stop=(d == NDC - 1),
            )
        vt = v_sb[tt]
        # ebqk = exp(0.125 * bq . k_raw[t])  (0.125 pre-folded into wkb)
        nc.scalar.activation(ebqk_sb[:, tt, :], psb[:], Exp)
        for h in range(HL):
            nc.scalar.activation(
                vt[:, h, 0:DK], psv[:, 64 * h : 64 * h + 64], Identity,
                scale=ebqk_sb[:, tt, h : h + 1],
            )
        nc.vector.tensor_copy(
            vt[:, :, DK : DK + 1].rearrange("p h o -> p (h o)"), ebqk_sb[:, tt, :]
        )

    bqkps_cm.__exit__(None, None, None)
    wvp_cm.__exit__(None, None, None)

    # wo loaded early (gpsimd queue), used by pair-3 interleaved O-proj
    wo_sb = wop.tile([NP, NPAIR, D], BF16, tag="wo")

    attnps_cm = tc.tile_pool(name="attnps", bufs=1, space="PSUM")
    attnps = attnps_cm.__enter__()

    def score_tile():
        return attnps.tile([NP, 2 * QB], F32, tag="s", bufs=2, name="s")

    def pv_tile(h):
        return attnps.tile([DK + 1, QB], F32, tag=f"pv{h}", bufs=1, name=f"pv{h}")

    # ---- Q/K projection steps: two 512-blocks per chunk load ----
    qt_t = [None] * NPAIR
    kt_t = [None] * NPAIR

    def alloc_qk(c):
        qt_t[c] = qkp.tile([NP, T], BF16, tag="qt", name=f"qt{c}")
        kt_t[c] = qkp.tile([NP, T], BF16, tag="kt", name=f"kt{c}")

    def emit_proj_step(c, which, tbp):
        slA = slice(2 * tbp * QB, (2 * tbp + 1) * QB)
        slB = slice((2 * tbp + 1) * QB, (2 * tbp + 2) * QB)
        w_sb = wq_t[c] if which == "q" else wk_t[c]
        dst = qt_t[c] if which == "q" else kt_t[c]
        psA = projps.tile([NP, QB], F32, tag="proj", name="psA")
        psB = projps.tile([NP, QB], F32, tag="proj", name="psB")
        for d in range(NDC):
            nc.tensor.matmul(
                psA[:], w_sb[:, d, :], xt_sb[:, d, slA],
                start=(d == 0), stop=(d == NDC - 1),
            )
            nc.tensor.matmul(
                psB[:], w_sb[:, d, :], xt_sb[:, d, slB],
                start=(d == 0), stop=(d == NDC - 1),
            )
        nc.vector.tensor_copy(dst[:, slA], psA[:])
        nc.vector.tensor_copy(dst[:, slB], psB[:])

    def proj_steps(c):
        for tbp in range(NTB // 2):
            yield ("proj", c, "q", tbp)
            yield ("proj", c, "k", tbp)

    # ---- O-proj step: both 512-halves per oat chunk load ----
    oat = [
        oatp.tile([NP, T], BF16, tag=f"oat{c}", name=f"oat{c}") for c in range(NPAIR)
    ]

    def emit_oproj_tt(tt, use_act):
        tsl = slice(tt * NP, (tt + 1) * NP)
        ya = projps.tile([NP, QB], F32, tag="proj", name="ya")
        yb = projps.tile([NP, QB], F32, tag="proj", name="yb")
        for cc in range(NPAIR):
            nc.tensor.matmul(
                ya[:], oat[cc][:, tsl], wo_sb[:, cc, 0:QB],
                start=(cc == 0), stop=(cc == NPAIR - 1),
            )
            nc.tensor.matmul(
                yb[:], oat[cc][:, tsl], wo_sb[:, cc, QB:D],
                start=(cc == 0), stop=(cc == NPAIR - 1),
            )
        for dh, yps in enumerate((ya, yb)):
            yst = smallp.tile([NP, QB], BF16, tag="ystage", bufs=4, name="yst")
            if use_act:
                nc.scalar.activation(yst[:], yps[:], Identity)
            else:
                nc.vector.tensor_copy(yst[:], yps[:])
            nc.gpsimd.dma_start(y[tsl, dh * QB : (dh + 1) * QB], yst[:])

    # pair 0 projections up front
    alloc_qk(0)
    for _, cc, which, tbp in proj_steps(0):
        emit_proj_step(cc, which, tbp)

    # ---- attention per pair ----
    for c in range(NPAIR):
        if c + 1 < NPAIR:
            emit_wqk_dma(c + 1)
            alloc_qk(c + 1)
            filler = proj_steps(c + 1)
            fill_total = 4
        else:
            # pair 3: interleave O-proj for q-blocks whose oat is complete
            filler = None
            fill_total = 0
        if c == 2:
            for cc in range(NPAIR):
                nc.gpsimd.dma_start(wo_sb[:, cc, :], wo[cc * NP : (cc + 1) * NP, :])
        qt, kt = qt_t[c], kt_t[c]
        nflr = 0
        kt_total = sum(4 * qb + 4 for qb in range(NTB))  # 40
        kt_seen = 0
        oproj_ready = []

        pvs = pvsp.tile([NP, NTB, QB], BF16, tag="pvs", name=f"pvs{c}")

        for qb in range(NTB):
            qsl0 = qb * QB
            nkt = 4 * qb + 4
            pv = [pv_tile(h) for h in range(2)]
            prev = None
            for kti in range(nkt):
                di = kti - 4 * qb
                o = max(di, 0) * NP
                sps = score_tile()
                for h in range(2):
                    nc.tensor.matmul(
                        sps[:, h * QB + o : (h + 1) * QB],
                        kt[64 * h : 64 * h + 64, kti * NP : (kti + 1) * NP],
                        qt[64 * h : 64 * h + 64, qsl0 + o : qsl0 + QB],
                        start=True, stop=True,
                        tile_position=(64 * h, 0),
                    )
                if prev is not None:
                    _emit_exp_pv(nc, prev, qb, etp, tri_sb, pv, nkt, v_sb, c)
                prev = (kti, o, sps)
                kt_seen += 1
                if filler is not None:
                    want = (kt_seen * fill_total) // kt_total
                    while nflr < want:
                        try:
                            _, cc, which, tbp = next(filler)
                        except StopIteration:
                            nflr = fill_total
                            break
                        emit_proj_step(cc, which, tbp)
                        nflr += 1
                elif oproj_ready and kti % 2 == 1:
                    emit_oproj_tt(oproj_ready.pop(0), use_act=False)
            _emit_exp_pv(nc, prev, qb, etp, tri_sb, pv, nkt, v_sb, c)

            # extract unnormalized out^T + per-qb division
            dcp = [None, None]
            for h in range(2):
                nc.vector.tensor_copy(pvs[64 * h : 64 * h + 64, qb, :], pv[h][0:DK, :])
                dcp[h] = dnp.tile([1, QB], F32, tag=f"dcp{h}", name=f"dcp{h}")
                nc.vector.tensor_copy(dcp[h][:], pv[h][DK : DK + 1, :])
            bc = projps.tile([NP, QB], F32, tag="proj")
            for h in range(2):
                dscr = dnp.tile([1, QB], F32, tag="dscr", name="dscr")
                nc.vector.reciprocal_approx_fast(dscr[:], dcp[h][:])
                recbf = dnp.tile([1, QB], BF16, tag="recbf", name="recbf")
                nc.vector.tensor_copy(recbf[:], dscr[:])
                nc.tensor.matmul(
                    bc[:], sel64[h][:], recbf[:], start=(h == 0), stop=(h == 1)
                )
            bcs = smallp.tile([NP, QB], BF16, tag="bcs")
            nc.vector.tensor_copy(bcs[:], bc[:])
            nc.vector.tensor_mul(
                oat[c][:, qb * QB : (qb + 1) * QB], pvs[:, qb, :], bcs[:]
            )
            if c == NPAIR - 1:
                oproj_ready.extend(range(4 * qb, 4 * qb + 4))

        if filler is not None:
            for _, cc, which, tbp in filler:
                emit_proj_step(cc, which, tbp)

    # remaining O-proj tiles (last q-block of pair 3)
    for tt in oproj_ready:
        emit_oproj_tt(tt, use_act=True)

    attnps_cm.__exit__(None, None, None)


def _emit_exp_pv(nc, prev, qb, etp, tri_sb, pv, nkt, v_sb, c):
    """one exp over both heads -> (triangle zero) -> 2 PV accumulates."""
    kti, o, sps = prev
    diag = kti >= 4 * qb
    et = etp.tile([NP, 2 * QB], BF16, tag="et", name="et")
    nc.scalar.activation(et[:, o : 2 * QB], sps[:, o : 2 * QB], Exp, scale=0.125)
    if diag:
        for h in range(2):
            nc.vector.tensor_mul(
                et[:, h * QB + o : h * QB + o + NP],
                et[:, h * QB + o : h * QB + o + NP],
                tri_sb[:],
            )
    for h in range(2):
        nc.tensor.matmul(
            pv[h][:, o:QB],
            v_sb[kti][:, 2 * c + h, 0 : DK + 1],
            et[:, h * QB + o : (h + 1) * QB],
            start=(kti == 0), stop=(kti == nkt - 1),
        )


def _install_ntff_hook_shim():
    """Provide the missing axon_hooks module so trace=True works under axon."""
    try:
        import sys
        import types

        if "antenv.axon_hooks" not in sys.modules:
            mod = types.ModuleType("antenv.axon_hooks")
            mod._hook = None
            mod.set_axon_ntff_profile_hook = lambda h: setattr(mod, "_hook", h)
            mod.get_axon_ntff_profile_hook = lambda: mod._hook
            sys.modules["antenv.axon_hooks"] = mod
            import antenv

            antenv.axon_hooks = mod
        from antenv.axon_hooks import (
            get_axon_ntff_profile_hook,
            set_axon_ntff_profile_hook,
        )

        if get_axon_ntff_profile_hook() is None:
            from trn_agent_boot.trn_boot import _ntff_profile_via_ctypes

            hook = _ntff_profile_via_ctypes("/opt/axon/libaxon_pjrt.so")
            if hook is not None:
                set_axon_ntff_profile_hook(hook)
    except Exception as e:  # noqa: BLE001
        print(f"ntff hook shim failed ({e}); running without trace")


def _bf(a: np.ndarray) -> np.ndarray:
    return np.ascontiguousarray(a, dtype=np.float32).astype(ml_dtypes.bfloat16)


def kernel(x, Wq, bq, Wk, bk, Wv, bv, Wo, bo):
    x = np.ascontiguousarray(np.asarray(x, dtype=np.float32))
    Wq, bq = np.asarray(Wq, np.float32), np.asarray(bq, np.float32)
    Wk, bk = np.asarray(Wk, np.float32), np.asarray(bk, np.float32)
    Wv, bv = np.asarray(Wv, np.float32), np.asarray(bv, np.float32)
    Wo, bo = np.asarray(Wo, np.float32), np.asarray(bo, np.float32)

    if "nc" not in _CACHE:
        _CACHE["nc"] = _build_nc()
    nc = _CACHE["nc"]

    kk = np.arange(NP)[:, None]
    qq = np.arange(NP)[None, :]
    tri_np = (qq >= kk).astype(np.float32)

    in_maps = []
    for core in range(8):
        b, hg = core // 2, core % 2
        cs = slice(hg * CD, (hg + 1) * CD)
        Wk_c = Wk[:, cs]
        bq_c = bq[cs]
        # wkb[:, h] = 0.125 * Wk_h @ bq_h  (per local head)
        wkb_np = np.stack(
            [
                0.125 * (Wk_c[:, h * DK : (h + 1) * DK] @ bq_c[h * DK : (h + 1) * DK])
                for h in range(HL)
            ],
            axis=1,
        )
        in_maps.append(
            {
                "xt": _bf(x[b].T),
                "wq": _bf(Wq[:, cs]),
                "wk": _bf(Wk_c),
                "wv": _bf(Wv[:, cs]),
                "wkb": _bf(wkb_np),
                "wo": _bf(Wo[cs, :]),
                "tri": _bf(tri_np),
            }
        )

    trace = bool(os.environ.get("KERNEL_TRACE"))
    if trace:
        _install_ntff_hook_shim()
    res = run_bass_kernel_spmd(nc, in_maps, core_ids=list(range(8)), trace=trace)
    _CACHE["last_results"] = res

    bo_eff = bo + bv @ Wo
    out = np.empty((B, T, D), dtype=np.float32)
    for b in range(B):
        out[b] = (
            res.results[2 * b]["y"].astype(np.float32)
            + res.results[2 * b + 1]["y"].astype(np.float32)
            + bo_eff
        )
    return out



# revision 16
# speedup vs baseline: 1.2073x; 1.2073x over previous
"""Causal self-attention (B=4, T=2048, D=1024, H=16) on 8 NeuronCores.

Sharding: core c handles batch b=c//2 and head-group hg=c%2 (8 of 16 heads).
Per core: column-parallel Wq/Wk/Wv (512 cols), row-parallel Wo (512 rows).
Host sums the two partial outputs per batch and adds bo_eff. No collectives.

Structure (v4):
  - x^T resident in SBUF (bf16), loaded once with block DMAs (sync queue).
  - bk drops (softmax invariance).  bv folds into bo on host.  bq is added
    during the Q-projection PSUM->SBUF copy as a per-partition scalar
    (tensor_scalar_add), so V needs no scaling and exp needs no bias.
  - V phase: 8 matmuls/tile + one PSUM->SBUF copy (alternating ACT/DVE) +
    a constant ones column (sumexp rides the PV matmul at M=65).
  - Scores: both heads write ONE [128,1024] 2-bank PSUM tile; one exp per
    step covers both heads; causal triangle zeroed post-exp on DVE.
  - Division per q-block: d rows -> [2,512]; one reciprocal_approx_fast;
    1/d broadcast to 128 partitions via gpsimd partition_broadcast (f32);
    one DVE multiply into oat.  bc selector matmuls eliminated.
  - Fine-grained PE fillers: projection / O-projection work is emitted in
    2-matmul quanta between attention steps so the ACT exp pipeline never
    starves and the PE never idles (idle resets the PE clock ramp).
  - O-proj interleaves into pair 3 per-chunk; final tiles split copies
    across ACT/DVE and y DMAs across gpsimd/sync queues.
"""

import os
from collections import deque
from contextlib import ExitStack

import ml_dtypes
import numpy as np

import concourse.bacc as bacc
import concourse.mybir as mybir
import concourse.tile as tile
from concourse.bass_utils import run_bass_kernel_spmd

B, T, D, H, DK = 4, 2048, 1024, 16, 64
HL = 8  # heads per core
CD = HL * DK  # 512 local channels
NP = 128  # partitions
QB = 512  # query block
NDC = D // NP  # 8 din chunks
NTT = T // NP  # 16 t-tiles
NTB = T // QB  # 4 t-blocks
NPAIR = HL // 2  # 4 head pairs
VW = DK + 2  # v row stride (64 data + 1 ones + 1 pad)
F32 = mybir.dt.float32
BF16 = mybir.dt.bfloat16
Exp = mybir.ActivationFunctionType.Exp
Identity = mybir.ActivationFunctionType.Identity

USE_PBCAST = False  # gpsimd partition_broadcast for 1/d; else selector matmul

_CACHE: dict = {}


def _build_nc():
    nc = bacc.Bacc("TRN2", target_bir_lowering=False, debug=False)
    xt = nc.dram_tensor("xt", [D, T], BF16, kind="ExternalInput")
    wq = nc.dram_tensor("wq", [D, CD], BF16, kind="ExternalInput")
    wk = nc.dram_tensor("wk", [D, CD], BF16, kind="ExternalInput")
    wv = nc.dram_tensor("wv", [D, CD], BF16, kind="ExternalInput")
    bqv = nc.dram_tensor("bqv", [NP, NPAIR], F32, kind="ExternalInput")
    wo = nc.dram_tensor("wo", [CD, D], BF16, kind="ExternalInput")
    tri = nc.dram_tensor("tri", [NP, NP], BF16, kind="ExternalInput")
    sel2d = None
    if not USE_PBCAST:
        sel2d = nc.dram_tensor("sel2d", [2, NP], BF16, kind="ExternalInput")
    y = nc.dram_tensor("y", [T, D], BF16, kind="ExternalOutput")

    with tile.TileContext(nc) as tc, ExitStack() as ctx:
        _body(nc, tc, ctx, xt, wq, wk, wv, bqv, wo, tri, sel2d, y)
    nc.compile()
    return nc


def _body(nc, tc, ctx, xt, wq, wk, wv, bqv, wo, tri, sel2d, y):
    const = ctx.enter_context(tc.tile_pool(name="const", bufs=1))
    xtp = ctx.enter_context(tc.tile_pool(name="xt", bufs=1))
    vpool = ctx.enter_context(tc.tile_pool(name="v", bufs=1))
    oatp = ctx.enter_context(tc.tile_pool(name="oat", bufs=1))
    wqkp = ctx.enter_context(tc.tile_pool(name="wqk", bufs=2))
    qkp = ctx.enter_context(tc.tile_pool(name="qk", bufs=2))
    etp = ctx.enter_context(tc.tile_pool(name="et", bufs=2))
    pvsp = ctx.enter_context(tc.tile_pool(name="pvs", bufs=2))
    dnp = ctx.enter_context(tc.tile_pool(name="dn", bufs=2))
    bcp = ctx.enter_context(tc.tile_pool(name="bc", bufs=2))
    smallp = ctx.enter_context(tc.tile_pool(name="small", bufs=2))
    wop = ctx.enter_context(tc.tile_pool(name="wop", bufs=1))
    wvp = ctx.enter_context(tc.tile_pool(name="wvp", bufs=1))
    # PSUM: proj(2 banks) + scores(2x2) + pv(2x1) = 8 banks
    projps = ctx.enter_context(tc.tile_pool(name="projps", bufs=2, space="PSUM"))

    # ---- constants ----
    tri_sb = const.tile([NP, NP], BF16, tag="tri")
    nc.scalar.dma_start(tri_sb[:], tri[:])
    bqv_sb = const.tile([NP, NPAIR], F32, tag="bqv")
    nc.scalar.dma_start(bqv_sb[:], bqv[:])
    sel2 = None
    if not USE_PBCAST:
        sel2 = [
            const.tile([1, NP], BF16, tag=f"sel2_{h}", name=f"sel2_{h}")
            for h in range(2)
        ]
        for h in range(2):
            nc.scalar.dma_start(sel2[h][:], sel2d[h : h + 1, :])
    warm = const.tile([1, 2], F32, tag="warm")
    nc.vector.memset(warm[:], 0.0)
    nc.scalar.activation(warm[:], warm[:], Exp)

    # ---- resident x^T (bf16): block DMAs, t-block major so V starts early
    xt_sb = xtp.tile([NP, NDC, T], BF16, tag="xt")
    for tb in range(NTB):
        bsl = slice(tb * QB, (tb + 1) * QB)
        for d in range(NDC):
            nc.sync.dma_start(xt_sb[:, d, bsl], xt[d * NP : (d + 1) * NP, bsl])

    # ---- V weights on the scalar queue (parallel with xt on sync) ----
    wv_sb = wvp.tile([NP, NDC, CD], BF16, tag="wv")
    for d in range(NDC):
        nc.scalar.dma_start(wv_sb[:, d, :], wv[d * NP : (d + 1) * NP, :])

    wq_t = [None] * NPAIR
    wk_t = [None] * NPAIR

    def emit_wqk_dma(c):
        wq_t[c] = wqkp.tile([NP, NDC, NP], BF16, tag="wqc", name="wqc")
        wk_t[c] = wqkp.tile([NP, NDC, NP], BF16, tag="wkc", name="wkc")
        for d in range(NDC):
            nc.gpsimd.dma_start(
                wq_t[c][:, d, :], wq[d * NP : (d + 1) * NP, c * NP : (c + 1) * NP]
            )
            nc.gpsimd.dma_start(
                wk_t[c][:, d, :], wk[d * NP : (d + 1) * NP, c * NP : (c + 1) * NP]
            )

    emit_wqk_dma(0)

    # ---- V tiles: ones column constant, data filled per t-tile ----
    v_sb = [
        vpool.tile([NP, HL, VW], BF16, tag=f"v{tt}", name=f"v{tt}")
        for tt in range(NTT)
    ]
    for tt in range(NTT):
        nc.vector.memset(v_sb[tt][:, :, DK : DK + 1], 1.0)

    # ---- V phase: plain projection + PSUM->SBUF copy ----
    for tt in range(NTT):
        tsl = slice(tt * NP, (tt + 1) * NP)
        psv = projps.tile([NP, CD], F32, tag="proj")
        for d in range(NDC):
            nc.tensor.matmul(
                psv[:], xt_sb[:, d, tsl], wv_sb[:, d, :],
                start=(d == 0), stop=(d == NDC - 1),
            )
        dst = v_sb[tt][:, :, 0:DK]
        src = psv[:].rearrange("p (h d) -> p h d", h=HL)
        if tt % 2 == 0:
            nc.scalar.activation(dst, src, Identity)
        else:
            nc.vector.tensor_copy(dst, src)

    # wo loaded on the (idle) sync queue before pair-2 attention
    wo_sb = wop.tile([NP, NPAIR, D], BF16, tag="wo")

    attnps_cm = tc.tile_pool(name="attnps", bufs=1, space="PSUM")
    attnps = attnps_cm.__enter__()

    def score_tile():
        return attnps.tile([NP, 2 * QB], F32, tag="s", bufs=2, name="s")

    def pv_tile(h):
        return attnps.tile([DK + 1, QB], F32, tag=f"pv{h}", bufs=1, name=f"pv{h}")

    # ---- Q/K projection quanta: 2 matmuls (or 2 copies) per yield ----
    qt_t = [None] * NPAIR
    kt_t = [None] * NPAIR

    def alloc_qk(c):
        qt_t[c] = qkp.tile([NP, T], BF16, tag="qt", name=f"qt{c}")
        kt_t[c] = qkp.tile([NP, T], BF16, tag="kt", name=f"kt{c}")

    def proj_quanta(c):
        for tbp in range(NTB // 2):
            for which in ("q", "k"):
                slA = slice(2 * tbp * QB, (2 * tbp + 1) * QB)
                slB = slice((2 * tbp + 1) * QB, (2 * tbp + 2) * QB)
                w_sb = wq_t[c] if which == "q" else wk_t[c]
                dst = qt_t[c] if which == "q" else kt_t[c]
                psA = projps.tile([NP, QB], F32, tag="proj", name="psA")
                psB = projps.tile([NP, QB], F32, tag="proj", name="psB")
                for d in range(NDC):
                    nc.tensor.matmul(
                        psA[:], w_sb[:, d, :], xt_sb[:, d, slA],
                        start=(d == 0), stop=(d == NDC - 1),
                    )
                    nc.tensor.matmul(
                        psB[:], w_sb[:, d, :], xt_sb[:, d, slB],
                        start=(d == 0), stop=(d == NDC - 1),
                    )
                    yield
                if which == "q":
                    nc.vector.tensor_scalar_add(dst[:, slA], psA[:], bqv_sb[:, c : c + 1])
                    nc.vector.tensor_scalar_add(dst[:, slB], psB[:], bqv_sb[:, c : c + 1])
                else:
                    nc.vector.tensor_copy(dst[:, slA], psA[:])
                    nc.vector.tensor_copy(dst[:, slB], psB[:])
                yield

    # ---- O-proj quanta ----
    oat = [
        oatp.tile([NP, T], BF16, tag=f"oat{c}", name=f"oat{c}") for c in range(NPAIR)
    ]
    ndma = [0]

    def oproj_quanta(tt, eng):
        tsl = slice(tt * NP, (tt + 1) * NP)
        ya = projps.tile([NP, QB], F32, tag="proj", name="ya")
        yb = projps.tile([NP, QB], F32, tag="proj", name="yb")
        for cc in range(NPAIR):
            nc.tensor.matmul(
                ya[:], oat[cc][:, tsl], wo_sb[:, cc, 0:QB],
                start=(cc == 0), stop=(cc == NPAIR - 1),
            )
            nc.tensor.matmul(
                yb[:], oat[cc][:, tsl], wo_sb[:, cc, QB:D],
                start=(cc == 0), stop=(cc == NPAIR - 1),
            )
            yield
        for dh, yps in enumerate((ya, yb)):
            yst = smallp.tile([NP, QB], BF16, tag="ystage", bufs=4, name="yst")
            if eng == "mixed":
                e = "scalar" if dh == 0 else "vector"
            else:
                e = eng
            if e == "scalar":
                nc.scalar.activation(yst[:], yps[:], Identity)
            else:
                nc.vector.tensor_copy(yst[:], yps[:])
            q = nc.gpsimd if ndma[0] % 2 == 0 else nc.sync
            ndma[0] += 1
            q.dma_start(y[tsl, dh * QB : (dh + 1) * QB], yst[:])
        yield

    # ---- division tail: broadcast 1/d and scale into oat ----
    def div_tail(c, qb, dscr, pvs):
        qsl = slice(qb * QB, (qb + 1) * QB)
        if USE_PBCAST:
            bcast = bcp.tile([NP, QB], F32, tag="bcast", name="bcast")
            nc.gpsimd.partition_broadcast(bcast[0:64, :], dscr[0][:], channels=64)
            nc.gpsimd.partition_broadcast(bcast[64:128, :], dscr[1][:], channels=64)
            yield
            nc.vector.tensor_mul(oat[c][:, qsl], pvs[:, qb, :], bcast[:])
            yield
        else:
            recbf = [None, None]
            for h in range(2):
                recbf[h] = dnp.tile([1, QB], BF16, tag=f"recbf{h}", name=f"recbf{h}")
                nc.vector.tensor_copy(recbf[h][:], dscr[h][:])
            bc = projps.tile([NP, QB], F32, tag="proj", name="bc")
            for h in range(2):
                nc.tensor.matmul(
                    bc[:], sel2[h][:], recbf[h][:],
                    start=(h == 0), stop=(h == 1),
                )
            yield
            nc.vector.tensor_mul(oat[c][:, qsl], pvs[:, qb, :], bc[:])
            yield

    # fillers: deque of (generator, on_done_callback)
    fillers = deque()

    def consume(budget):
        done = 0
        while done < budget and fillers:
            g, cb = fillers[0]
            try:
                next(g)
                done += 1
            except StopIteration:
                fillers.popleft()
                if cb is not None:
                    cb()
        return done

    # pair 0 projections up front
    alloc_qk(0)
    for _ in proj_quanta(0):
        pass

    oproj_emitted = [0]

    # ---- attention per pair ----
    for c in range(NPAIR):
        if c + 1 < NPAIR:
            emit_wqk_dma(c + 1)
            alloc_qk(c + 1)
            fillers.append((proj_quanta(c + 1), None))
        if c == 2:
            for cc in range(NPAIR):
                nc.sync.dma_start(wo_sb[:, cc, :], wo[cc * NP : (cc + 1) * NP, :])
        qt, kt = qt_t[c], kt_t[c]
        kt_total = sum(4 * qb + 4 for qb in range(NTB))  # 40
        kt_seen = 0
        nflr = 0
        # pairs 0-2: ~44 quanta (36 proj + div tails); pair 3: oproj-driven
        fill_total = 46 if c < NPAIR - 1 else 200

        pvs = pvsp.tile([NP, NTB, QB], BF16, tag="pvs", name=f"pvs{c}")

        def make_oproj_adder(lo, hi):
            def add():
                for tt in range(lo, hi):
                    fillers.append((oproj_quanta(tt, "vector"), None))
                    oproj_emitted[0] += 1
            return add

        for qb in range(NTB):
            qsl0 = qb * QB
            nkt = 4 * qb + 4
            pv = [pv_tile(h) for h in range(2)]
            prev = None
            for kti in range(nkt):
                di = kti - 4 * qb
                o = max(di, 0) * NP
                sps = score_tile()
                for h in range(2):
                    # head 1 packs left: [QB : 2QB-o] so the exp region is
                    # contiguous ([o : 2QB-o]) and o columns shorter
                    lo = o if h == 0 else QB
                    nc.tensor.matmul(
                        sps[:, lo : lo + QB - o],
                        kt[64 * h : 64 * h + 64, kti * NP : (kti + 1) * NP],
                        qt[64 * h : 64 * h + 64, qsl0 + o : qsl0 + QB],
                        start=True, stop=True,
                        tile_position=(64 * h, 0),
                    )
                kt_seen += 1
                want = (kt_seen * fill_total) // kt_total
                nflr += consume(max(0, want - nflr))
                if prev is not None:
                    _emit_exp_pv(nc, prev, qb, etp, tri_sb, pv, nkt, v_sb, c)
                prev = (kti, o, sps)
            _emit_exp_pv(nc, prev, qb, etp, tri_sb, pv, nkt, v_sb, c)

            # extract unnormalized out^T + start division
            for h in range(2):
                nc.vector.tensor_copy(pvs[64 * h : 64 * h + 64, qb, :], pv[h][0:DK, :])
            dscr = [None, None]
            for h in range(2):
                dcp = dnp.tile([1, QB], F32, tag=f"dcp{h}", name=f"dcp{h}")
                nc.vector.tensor_copy(dcp[:], pv[h][DK : DK + 1, :])
                dscr[h] = dnp.tile([1, QB], F32, tag=f"dscr{h}", name=f"dscr{h}")
                nc.vector.reciprocal_approx_fast(dscr[h][:], dcp[:])
            cb = None
            if c == NPAIR - 1:
                cb = make_oproj_adder(4 * qb, 4 * qb + 4)
            if qb < NTB - 1 or c < NPAIR - 1:
                fillers.appendleft((div_tail(c, qb, dscr, pvs), cb))
            else:
                # very last q-block: emit immediately
                for _ in div_tail(c, qb, dscr, pvs):
                    pass
                if cb is not None:
                    cb()

        if c < NPAIR - 1:
            # finish next-pair projections and pending div tails (no PE work
            # in div tails under USE_PBCAST, so this cannot stall the PE)
            consume(1000000)

    # remaining O-proj tiles
    for tt in range(4 * (NTB - 1), NTT):
        if oproj_emitted[0] < NTT:
            fillers.append((oproj_quanta(tt, "mixed"), None))
            oproj_emitted[0] += 1
    consume(1000000)

    attnps_cm.__exit__(None, None, None)


def _emit_exp_pv(nc, prev, qb, etp, tri_sb, pv, nkt, v_sb, c):
    """one exp over both heads -> (triangle zero) -> 2 PV accumulates."""
    kti, o, sps = prev
    diag = kti >= 4 * qb
    et = etp.tile([NP, 2 * QB], BF16, tag="et", name="et")
    nc.scalar.activation(
        et[:, o : 2 * QB - o], sps[:, o : 2 * QB - o], Exp, scale=0.125
    )
    if diag:
        for h in range(2):
            lo = o if h == 0 else QB
            nc.vector.tensor_mul(
                et[:, lo : lo + NP], et[:, lo : lo + NP], tri_sb[:]
            )
    for h in range(2):
        lo = o if h == 0 else QB
        nc.tensor.matmul(
            pv[h][:, o:QB],
            v_sb[kti][:, 2 * c + h, 0 : DK + 1],
            et[:, lo : lo + QB - o],
            start=(kti == 0), stop=(kti == nkt - 1),
        )


def _install_ntff_hook_shim():
    """Provide the missing axon_hooks module so trace=True works under axon."""
    try:
        import sys
        import types

        if "antenv.axon_hooks" not in sys.modules:
            mod = types.ModuleType("antenv.axon_hooks")
            mod._hook = None
            mod.set_axon_ntff_profile_hook = lambda h: setattr(mod, "_hook", h)
            mod.get_axon_ntff_profile_hook = lambda: mod._hook
            sys.modules["antenv.axon_hooks"] = mod
            import antenv

            antenv.axon_hooks = mod
        from antenv.axon_hooks import (
            get_axon_ntff_profile_hook,
            set_axon_ntff_profile_hook,
        )

        if get_axon_ntff_profile_hook() is None:
            from trn_agent_boot.trn_boot import _ntff_profile_via_ctypes

            hook = _ntff_profile_via_ctypes("/opt/axon/libaxon_pjrt.so")
            if hook is not None:
                set_axon_ntff_profile_hook(hook)
    except Exception as e:  # noqa: BLE001
        print(f"ntff hook shim failed ({e}); running without trace")


def _bf(a: np.ndarray) -> np.ndarray:
    return np.ascontiguousarray(a, dtype=np.float32).astype(ml_dtypes.bfloat16)


def make_in_maps(x, Wq, bq, Wk, Wv, Wo):
    kk = np.arange(NP)[:, None]
    qq = np.arange(NP)[None, :]
    tri_np = (qq >= kk).astype(np.float32)
    in_maps = []
    for core in range(8):
        b, hg = core // 2, core % 2
        cs = slice(hg * CD, (hg + 1) * CD)
        bqv_np = np.ascontiguousarray(
            bq[cs].reshape(NPAIR, NP).T, dtype=np.float32
        )
        in_maps.append(
            {
                "xt": _bf(x[b].T),
                "wq": _bf(Wq[:, cs]),
                "wk": _bf(Wk[:, cs]),
                "wv": _bf(Wv[:, cs]),
                "bqv": bqv_np,
                "wo": _bf(Wo[cs, :]),
                "tri": _bf(tri_np),
            }
        )
        if not USE_PBCAST:
            sel2_np = np.zeros((2, NP), np.float32)
            sel2_np[0, 0:64] = 1.0
            sel2_np[1, 64:128] = 1.0
            in_maps[-1]["sel2d"] = _bf(sel2_np)
    return in_maps


def kernel(x, Wq, bq, Wk, bk, Wv, bv, Wo, bo):
    x = np.ascontiguousarray(np.asarray(x, dtype=np.float32))
    Wq, bq = np.asarray(Wq, np.float32), np.asarray(bq, np.float32)
    Wk = np.asarray(Wk, np.float32)
    Wv, bv = np.asarray(Wv, np.float32), np.asarray(bv, np.float32)
    Wo, bo = np.asarray(Wo, np.float32), np.asarray(bo, np.float32)

    if "nc" not in _CACHE:
        _CACHE["nc"] = _build_nc()
    nc = _CACHE["nc"]

    in_maps = make_in_maps(x, Wq, bq, Wk, Wv, Wo)

    trace = bool(os.environ.get("KERNEL_TRACE"))
    if trace:
        _install_ntff_hook_shim()
    res = run_bass_kernel_spmd(nc, in_maps, core_ids=list(range(8)), trace=trace)
    _CACHE["last_results"] = res

    bo_eff = bo + bv @ Wo
    out = np.empty((B, T, D), dtype=np.float32)
    for b in range(B):
        out[b] = (
            res.results[2 * b]["y"].astype(np.float32)
            + res.results[2 * b + 1]["y"].astype(np.float32)
            + bo_eff
        )
    return out
